# revision 5
# baseline (speedup 1.0000x reference)
"""CrossAttend Trainium2 kernel: 8-way data-parallel over batch.

Full inputs arrive here; we shard batch B=16 across 8 NeuronCores
(2 batch elements per core), replicate the 512x512 projection weights,
run one SPMD Bass/Tile kernel, and concatenate the per-core outputs.

v3 design:
  - All tensors/matmuls bf16 (1 cycle/row on PE, FWL-compatible);
    validated rel err ~7e-3 vs the 2e-2 gate.
  - q/opp transposed to [H, L] on the HOST (no PE transposes).
  - bk drops (softmax shift-invariance); qpk := qp @ Wk shared by both
    attentions.
  - Rowsums OFF the PE entirely: DVE accumulates R = sum_ko pexp[:,ko,:]
    chunk-by-chunk during the exp phase (f32), then one GpSimd
    partition-axis reduce -> [1, L]; the softmax division happens on the
    HOST (device ships unnormalized PV bf16 + rowsums f32).
  - Self-attn diagonal zeroed after exp via affine_select on the eight
    128x128 diagonal blocks.
  - Startup: PE-warmup matmuls + dummy activation run during the input
    DMA window (clock ramp + act table preload); weights DMA'd first on
    the sync queue, input loads split in halves; stores go on the
    scalar queue so they never block loads.
"""

import contextlib
import math

import numpy as np
import ml_dtypes

import concourse.bass as bass
import concourse.bass_isa as bass_isa
import concourse.mybir as mybir
import concourse.tile as tile
from concourse import bacc
from concourse.bass_utils import run_bass_kernel_spmd

F32 = mybir.dt.float32
BF = mybir.dt.bfloat16

B = 16
H = 512
L = 1024
P = 128
NCORES = 8
BPC = B // NCORES   # batch elements per core
HT = H // P         # 4 h-tiles
LT = L // P         # 8 l-tiles
QC = L // 512       # 2 q-chunks of 512
SCALE = 1.0 / math.sqrt(H)


def _build_core_kernel(ctx, tc, ins, outs):
    nc = tc.nc
    AF = mybir.ActivationFunctionType

    qT_d = ins["qT"]        # [BPC, H, L] bf16 (host-pre-transposed)
    oppT_d = ins["oppT"]    # [BPC, H, L] bf16
    self_d = outs["self_pv"]
    oout_d = outs["opp_pv"]
    rs_d = outs["rs"]       # [BPC, 2, L] f32

    wpool = ctx.enter_context(tc.tile_pool(name="w", bufs=1))
    xpool = ctx.enter_context(tc.tile_pool(name="x", bufs=4))
    mpool = ctx.enter_context(tc.tile_pool(name="m", bufs=4))
    vpool = ctx.enter_context(tc.tile_pool(name="v", bufs=4))
    ppool = ctx.enter_context(tc.tile_pool(name="P", bufs=2))
    rapool = ctx.enter_context(tc.tile_pool(name="ra", bufs=2))
    opool = ctx.enter_context(tc.tile_pool(name="o", bufs=6))
    rpool = ctx.enter_context(tc.tile_pool(name="r", bufs=4))
    ps_mm = ctx.enter_context(tc.tile_pool(name="psmm", bufs=5, space="PSUM"))
    ps_pv = ctx.enter_context(tc.tile_pool(name="pspv", bufs=3, space="PSUM"))

    # --- PE warmup + act table preload (runs while input DMAs stream) ---
    scratch = wpool.tile([P, 512], BF, tag="scr")
    nc.vector.memset(scratch[:], 0.0)
    for _ in range(10):
        psw = ps_mm.tile([P, 512], F32, tag="psmm")
        nc.tensor.matmul(psw[:], lhsT=scratch[:, 0:P], rhs=scratch[:],
                         start=True, stop=True)
    nc.scalar.activation(scratch[:, 0:1], scratch[:, 0:1], AF.Exp, scale=1.0)

    # --- constants; critical-path-ordered loads on the sync queue ---
    wq = wpool.tile([P, HT, H], BF, tag="wq")
    nc.sync.dma_start(wq[:], ins["WqT"].rearrange("(ko ki) m -> ki ko m", ki=P))
    bq = wpool.tile([P, HT], F32, tag="bq")
    nc.sync.dma_start(bq[:], ins["bq_p"][:])
    bvb = wpool.tile([P, H], F32, tag="bvb")
    nc.sync.dma_start(bvb[:], ins["bv_b"][:])

    def load_x(src_d, b):
        x = xpool.tile([P, HT, L], BF, tag="x")
        r = src_d[b, :, :].rearrange("(ko ki) l -> ki ko l", ki=P)
        nc.sync.dma_start(x[:, :, 0:512], r[:, :, 0:512])
        nc.sync.dma_start(x[:, :, 512:L], r[:, :, 512:L])
        return x

    q0 = load_x(qT_d, 0)
    wk = wpool.tile([P, HT, H], BF, tag="wk")
    nc.sync.dma_start(wk[:], ins["Wk"].rearrange("(ko ki) m -> ki ko m", ki=P))
    wv = wpool.tile([P, HT, H], BF, tag="wv")
    nc.sync.dma_start(wv[:], ins["WvT"].rearrange("(ko ki) m -> ki ko m", ki=P))
    o0 = load_x(oppT_d, 0)
    q1 = load_x(qT_d, 1)
    o1 = load_x(oppT_d, 1)
    xs = [(q0, o0), (q1, o1)]

    def proj_T(srcT, w, bias=None):
        """dst[h_out-part, l] = sum_hin w[hin, hout-tile].T @ srcT[hin, l]."""
        dst = mpool.tile([P, HT, L], BF, tag="m")
        for ht in range(HT):
            for qc in range(QC):
                ps = ps_mm.tile([P, 512], F32, tag="psmm")
                for hc in range(HT):
                    nc.tensor.matmul(
                        ps[:],
                        lhsT=w[:, hc, P * ht:P * (ht + 1)],
                        rhs=srcT[:, hc, 512 * qc:512 * (qc + 1)],
                        start=(hc == 0),
                        stop=(hc == HT - 1),
                    )
                d = dst[:, ht, 512 * qc:512 * (qc + 1)]
                if bias is not None:
                    nc.scalar.activation(d, ps[:], AF.Identity,
                                         bias=bias[:, ht:ht + 1], scale=1.0)
                else:
                    nc.vector.tensor_copy(d, ps[:])
        return dst

    def proj_nat(srcT, w_rhs):
        """dst[l-part, h_out] = srcT[hin, l-tile].T @ w_rhs[hin, hout] + bv."""
        dst = vpool.tile([P, LT, H], BF, tag="v")
        for lt in range(LT):
            ps = ps_mm.tile([P, 512], F32, tag="psmm")
            for hc in range(HT):
                nc.tensor.matmul(
                    ps[:],
                    lhsT=srcT[:, hc, P * lt:P * (lt + 1)],
                    rhs=w_rhs[:, hc, :],
                    start=(hc == 0),
                    stop=(hc == HT - 1),
                )
            nc.vector.tensor_tensor(dst[:, lt, :], ps[:], bvb[:],
                                    mybir.AluOpType.add)
        return dst

    def attn(lhsT_T, qpkT, vv, out_d, b, ai, masked):
        """pexp[k, q] = exp(scale * lhsT_T.T @ qpkT); rowsums via DVE+GpSimd;
        unnormalized out = pexp.T @ v -> HBM (host divides)."""
        pexp = ppool.tile([P, LT, L], BF, tag="P")
        racc = rapool.tile([P, L], F32, tag="ra")
        for qc in range(QC):
            cs = slice(512 * qc, 512 * (qc + 1))
            for ko in range(LT):
                ps = ps_mm.tile([P, 512], F32, tag="psmm")
                for hc in range(HT):
                    nc.tensor.matmul(
                        ps[:],
                        lhsT=lhsT_T[:, hc, P * ko:P * (ko + 1)],
                        rhs=qpkT[:, hc, cs],
                        start=(hc == 0),
                        stop=(hc == HT - 1),
                    )
                nc.scalar.activation(pexp[:, ko, cs], ps[:], AF.Exp, scale=SCALE)
                if masked and ko // 4 == qc:
                    d = pexp[:, ko, P * ko:P * (ko + 1)]
                    nc.gpsimd.affine_select(
                        out=d, in_=d,
                        compare_op=mybir.AluOpType.not_equal,
                        fill=0.0, base=0,
                        pattern=[[-1, P]], channel_multiplier=1,
                    )
                if ko == 0:
                    nc.vector.tensor_copy(racc[:, cs], pexp[:, 0, cs])
                else:
                    nc.vector.tensor_tensor(racc[:, cs], racc[:, cs],
                                            pexp[:, ko, cs],
                                            mybir.AluOpType.add)
        rout = rpool.tile([P, L], F32, tag="r")
        nc.gpsimd.partition_all_reduce(rout[:], racc[:], P,
                                       bass_isa.ReduceOp.add)
        nc.scalar.dma_start(rs_d[b, ai:ai + 1, :], rout[0:1, :])
        # PV (unnormalized)
        for qo in range(LT):
            ps = ps_pv.tile([P, 512], F32, tag="pspv")
            for ko in range(LT):
                nc.tensor.matmul(
                    ps[:], lhsT=pexp[:, ko, P * qo:P * (qo + 1)],
                    rhs=vv[:, ko, :],
                    start=(ko == 0), stop=(ko == LT - 1),
                )
            ot = opool.tile([P, H], BF, tag="o")
            nc.scalar.activation(ot[:], ps[:], AF.Copy)
            nc.scalar.dma_start(out_d[b, P * qo:P * (qo + 1), :], ot[:])

    for b in range(BPC):
        qTt, oTt = xs[b]
        qpT = proj_T(qTt, wq, bias=bq)
        qpkT = proj_T(qpT, wk)
        vv = proj_nat(qpT, wv)
        ovv = proj_nat(oTt, wv)
        attn(qpT, qpkT, vv, self_d, b, 0, masked=True)
        attn(oTt, qpkT, ovv, oout_d, b, 1, masked=False)


_NC_CACHE = None


def _get_module():
    global _NC_CACHE
    if _NC_CACHE is not None:
        return _NC_CACHE
    nc = bacc.Bacc(None, target_bir_lowering=False, debug=False)
    ins = {
        "qT": nc.dram_tensor("qT", [BPC, H, L], BF, kind="ExternalInput").ap(),
        "oppT": nc.dram_tensor("oppT", [BPC, H, L], BF, kind="ExternalInput").ap(),
        "WqT": nc.dram_tensor("WqT", [H, H], BF, kind="ExternalInput").ap(),
        "Wk": nc.dram_tensor("Wk", [H, H], BF, kind="ExternalInput").ap(),
        "WvT": nc.dram_tensor("WvT", [H, H], BF, kind="ExternalInput").ap(),
        "bq_p": nc.dram_tensor("bq_p", [P, HT], F32, kind="ExternalInput").ap(),
        "bv_b": nc.dram_tensor("bv_b", [P, H], F32, kind="ExternalInput").ap(),
    }
    outs = {
        "self_pv": nc.dram_tensor("self_pv", [BPC, L, H], BF,
                                  kind="ExternalOutput").ap(),
        "opp_pv": nc.dram_tensor("opp_pv", [BPC, L, H], BF,
                                 kind="ExternalOutput").ap(),
        "rs": nc.dram_tensor("rs", [BPC, 2, L], F32,
                             kind="ExternalOutput").ap(),
    }
    with tile.TileContext(nc) as tc:
        with contextlib.ExitStack() as ctx:
            _build_core_kernel(ctx, tc, ins, outs)
    nc.compile()
    _NC_CACHE = nc
    return nc


def kernel(q, opp, Wq, bq, Wk, bk, Wv, bv):
    q = np.asarray(q, dtype=np.float32)
    opp = np.asarray(opp, dtype=np.float32)
    Wq = np.asarray(Wq, dtype=np.float32)
    Wk = np.asarray(Wk, dtype=np.float32)
    Wv = np.asarray(Wv, dtype=np.float32)
    bq = np.asarray(bq, dtype=np.float32)
    bv = np.asarray(bv, dtype=np.float32)
    # bk is mathematically irrelevant (softmax shift-invariance); unused.

    bf = ml_dtypes.bfloat16
    qT = np.ascontiguousarray(q.transpose(0, 2, 1)).astype(bf)    # [B, H, L]
    oppT = np.ascontiguousarray(opp.transpose(0, 2, 1)).astype(bf)
    shared = {
        "WqT": np.ascontiguousarray(Wq.T).astype(bf),
        "Wk": np.ascontiguousarray(Wk).astype(bf),
        "WvT": np.ascontiguousarray(Wv.T).astype(bf),
        "bq_p": np.ascontiguousarray(bq.reshape(HT, P).T),
        "bv_b": np.ascontiguousarray(np.tile(bv, (P, 1))),
    }
    in_maps = []
    for c in range(NCORES):
        sl = slice(c * BPC, (c + 1) * BPC)
        in_maps.append({
            "qT": np.ascontiguousarray(qT[sl]),
            "oppT": np.ascontiguousarray(oppT[sl]),
            **shared,
        })

    nc = _get_module()
    res = run_bass_kernel_spmd(nc, in_maps, core_ids=list(range(NCORES)))
    self_pv = np.concatenate([r["self_pv"] for r in res.results], axis=0)
    opp_pv = np.concatenate([r["opp_pv"] for r in res.results], axis=0)
    rs = np.concatenate([r["rs"] for r in res.results], axis=0)  # [B, 2, L]
    self_out = self_pv.astype(np.float32) / rs[:, 0, :, None]
    opp_out = opp_pv.astype(np.float32) / rs[:, 1, :, None]
    return (self_out, opp_out)


# revision 13
# speedup vs baseline: 1.1365x; 1.1365x over previous
"""CrossAttend Trainium2 kernel: 8-way data-parallel over batch.

Full inputs arrive here; we shard batch B=16 across 8 NeuronCores
(2 batch elements per core), replicate the 512x512 projection weights,
run one SPMD Bass/Tile kernel, and concatenate the per-core outputs.

v3 design:
  - All tensors/matmuls bf16 (1 cycle/row on PE, FWL-compatible);
    validated rel err ~7e-3 vs the 2e-2 gate.
  - q/opp transposed to [H, L] on the HOST (no PE transposes).
  - bk drops (softmax shift-invariance); qpk := qp @ Wk shared by both
    attentions.
  - Rowsums nearly free: DVE accumulates R = sum_ko pexp[:,ko,:] (bf16)
    chunk-by-chunk during the exp phase, then two tiny ones-stationary
    PE matmuls partition-reduce R -> [1, L]; the softmax division
    happens on the HOST (device ships unnormalized PV bf16 + rowsums f32).
  - Self-attn diagonal zeroed after exp via affine_select on the eight
    128x128 diagonal blocks.
  - Startup: PE-warmup matmuls + dummy activation run during the input
    DMA window (clock ramp + act table preload); weights DMA'd first on
    the sync queue, input loads split in halves; stores go on the
    scalar queue so they never block loads.
"""

import contextlib
import math

import numpy as np
import ml_dtypes

import concourse.bass as bass
import concourse.mybir as mybir
import concourse.tile as tile
from concourse import bacc
from concourse.bass_utils import run_bass_kernel_spmd

F32 = mybir.dt.float32
BF = mybir.dt.bfloat16

B = 16
H = 512
L = 1024
P = 128
NCORES = 8
BPC = B // NCORES   # batch elements per core
HT = H // P         # 4 h-tiles
LT = L // P         # 8 l-tiles
QC = L // 512       # 2 q-chunks of 512
SCALE = 1.0 / math.sqrt(H)


def _build_core_kernel(ctx, tc, ins, outs):
    nc = tc.nc
    AF = mybir.ActivationFunctionType

    qT_d = ins["qT"]        # [BPC, H, L] bf16 (host-pre-transposed)
    oppT_d = ins["oppT"]    # [BPC, H, L] bf16
    self_d = outs["self_pv"]
    oout_d = outs["opp_pv"]
    rs_d = outs["rs"]       # [BPC, 2, L] f32

    wpool = ctx.enter_context(tc.tile_pool(name="w", bufs=1))
    xpool = ctx.enter_context(tc.tile_pool(name="x", bufs=4))
    mpool = ctx.enter_context(tc.tile_pool(name="m", bufs=4))
    vpool = ctx.enter_context(tc.tile_pool(name="v", bufs=4))
    ppool = ctx.enter_context(tc.tile_pool(name="P", bufs=2))
    rapool = ctx.enter_context(tc.tile_pool(name="ra", bufs=2))
    opool = ctx.enter_context(tc.tile_pool(name="o", bufs=6))
    rpool = ctx.enter_context(tc.tile_pool(name="r", bufs=4))
    ps_mm = ctx.enter_context(tc.tile_pool(name="psmm", bufs=4, space="PSUM"))
    ps_pv = ctx.enter_context(tc.tile_pool(name="pspv", bufs=2, space="PSUM"))
    ps_rs = ctx.enter_context(tc.tile_pool(name="psrs", bufs=2, space="PSUM"))

    # --- PE warmup + act table preload (runs while input DMAs stream) ---
    scratch = wpool.tile([P, 512], BF, tag="scr")
    nc.vector.memset(scratch[:], 0.0)
    for _ in range(6):
        psw = ps_mm.tile([P, P], F32, tag="psmm")
        nc.tensor.matmul(psw[:], lhsT=scratch[:, 0:P], rhs=scratch[:, 0:P],
                         start=True, stop=True)
    nc.scalar.activation(scratch[:, 0:1], scratch[:, 0:1], AF.Exp, scale=1.0)
    ones = wpool.tile([P, 1], BF, tag="ones")
    nc.vector.memset(ones[:], 1.0)

    # --- constants; critical-path loads on the sync ring, rest on scalar ---
    wq = wpool.tile([P, HT, H], BF, tag="wq")
    nc.sync.dma_start(wq[:], ins["WqT"].rearrange("(ko ki) m -> ki ko m", ki=P))
    bq = wpool.tile([P, HT], F32, tag="bq")
    nc.scalar.dma_start(bq[:], ins["bq_p"][:])
    bvb = wpool.tile([P, H], F32, tag="bvb")
    nc.scalar.dma_start(bvb[:], ins["bv_b"][:])

    def load_x(src_d, b, eng):
        x = xpool.tile([P, HT, L], BF, tag="x")
        r = src_d[b, :, :].rearrange("(ko ki) l -> ki ko l", ki=P)
        eng.dma_start(x[:, :, 0:512], r[:, :, 0:512])
        eng.dma_start(x[:, :, 512:L], r[:, :, 512:L])
        return x

    q0 = load_x(qT_d, 0, nc.sync)
    wk = wpool.tile([P, HT, H], BF, tag="wk")
    nc.scalar.dma_start(wk[:], ins["Wk"].rearrange("(ko ki) m -> ki ko m", ki=P))
    wv = wpool.tile([P, HT, H], BF, tag="wv")
    nc.scalar.dma_start(wv[:], ins["WvT"].rearrange("(ko ki) m -> ki ko m", ki=P))
    o0 = load_x(oppT_d, 0, nc.sync)
    q1 = load_x(qT_d, 1, nc.scalar)
    o1 = load_x(oppT_d, 1, nc.scalar)
    xs = [(q0, o0), (q1, o1)]

    def proj_T(srcT, w, bias=None):
        """dst[h_out-part, l] = sum_hin w[hin, hout-tile].T @ srcT[hin, l]."""
        dst = mpool.tile([P, HT, L], BF, tag="m")
        for ht in range(HT):
            for qc in range(QC):
                ps = ps_mm.tile([P, 512], F32, tag="psmm")
                for hc in range(HT):
                    nc.tensor.matmul(
                        ps[:],
                        lhsT=w[:, hc, P * ht:P * (ht + 1)],
                        rhs=srcT[:, hc, 512 * qc:512 * (qc + 1)],
                        start=(hc == 0),
                        stop=(hc == HT - 1),
                    )
                d = dst[:, ht, 512 * qc:512 * (qc + 1)]
                if bias is not None:
                    nc.scalar.activation(d, ps[:], AF.Identity,
                                         bias=bias[:, ht:ht + 1], scale=1.0)
                else:
                    nc.vector.tensor_copy(d, ps[:])
        return dst

    def proj_nat(srcT, w_rhs):
        """dst[l-part, h_out] = srcT[hin, l-tile].T @ w_rhs[hin, hout] + bv."""
        dst = vpool.tile([P, LT, H], BF, tag="v")
        for lt in range(LT):
            ps = ps_mm.tile([P, 512], F32, tag="psmm")
            for hc in range(HT):
                nc.tensor.matmul(
                    ps[:],
                    lhsT=srcT[:, hc, P * lt:P * (lt + 1)],
                    rhs=w_rhs[:, hc, :],
                    start=(hc == 0),
                    stop=(hc == HT - 1),
                )
            nc.vector.tensor_tensor(dst[:, lt, :], ps[:], bvb[:],
                                    mybir.AluOpType.add)
        return dst

    def attn(lhsT_T, qpkT, vv, out_d, b, ai, masked):
        """pexp[k, q] = exp(scale * lhsT_T.T @ qpkT); rowsums via DVE+GpSimd;
        unnormalized out = pexp.T @ v -> HBM (host divides)."""
        pexp = ppool.tile([P, LT, L], BF, tag="P")
        racc = rapool.tile([P, L], BF, tag="ra")
        for qc in range(QC):
            cs = slice(512 * qc, 512 * (qc + 1))
            for ko in range(LT):
                ps = ps_mm.tile([P, 512], F32, tag="psmm")
                for hc in range(HT):
                    nc.tensor.matmul(
                        ps[:],
                        lhsT=lhsT_T[:, hc, P * ko:P * (ko + 1)],
                        rhs=qpkT[:, hc, cs],
                        start=(hc == 0),
                        stop=(hc == HT - 1),
                    )
                nc.scalar.activation(pexp[:, ko, cs], ps[:], AF.Exp, scale=SCALE)
                if masked and ko // 4 == qc:
                    d = pexp[:, ko, P * ko:P * (ko + 1)]
                    nc.gpsimd.affine_select(
                        out=d, in_=d,
                        compare_op=mybir.AluOpType.not_equal,
                        fill=0.0, base=0,
                        pattern=[[-1, P]], channel_multiplier=1,
                    )
                if ko == 0:
                    nc.vector.tensor_copy(racc[:, cs], pexp[:, 0, cs])
                else:
                    nc.vector.tensor_tensor(racc[:, cs], racc[:, cs],
                                            pexp[:, ko, cs],
                                            mybir.AluOpType.add)
        # PV (unnormalized)
        for qo in range(LT):
            ps = ps_pv.tile([P, 512], F32, tag="pspv")
            for ko in range(LT):
                nc.tensor.matmul(
                    ps[:], lhsT=pexp[:, ko, P * qo:P * (qo + 1)],
                    rhs=vv[:, ko, :],
                    start=(ko == 0), stop=(ko == LT - 1),
                )
            ot = opool.tile([P, H], BF, tag="o")
            nc.scalar.activation(ot[:], ps[:], AF.Copy)
            nc.sync.dma_start(out_d[b, P * qo:P * (qo + 1), :], ot[:])
        # rowsum partition-reduce; emitted after PV so the racc DVE chain
        # has the whole PV phase to finish (never stalls the PE)
        rs_sb = rpool.tile([1, L], F32, tag="r")
        for qc in range(QC):
            psr = ps_rs.tile([1, 512], F32, tag="psrs")
            nc.tensor.matmul(psr[:], lhsT=ones[:, 0:1],
                             rhs=racc[:, 512 * qc:512 * (qc + 1)],
                             start=True, stop=True)
            nc.vector.tensor_copy(rs_sb[0:1, 512 * qc:512 * (qc + 1)], psr[:])
        nc.sync.dma_start(rs_d[b, ai:ai + 1, :], rs_sb[:])

    for b in range(BPC):
        qTt, oTt = xs[b]
        qpT = proj_T(qTt, wq, bias=bq)
        qpkT = proj_T(qpT, wk)
        vv = proj_nat(qpT, wv)
        ovv = proj_nat(oTt, wv)
        attn(qpT, qpkT, vv, self_d, b, 0, masked=True)
        attn(oTt, qpkT, ovv, oout_d, b, 1, masked=False)


_NC_CACHE = None


def _get_module():
    global _NC_CACHE
    if _NC_CACHE is not None:
        return _NC_CACHE
    nc = bacc.Bacc(None, target_bir_lowering=False, debug=False)
    ins = {
        "qT": nc.dram_tensor("qT", [BPC, H, L], BF, kind="ExternalInput").ap(),
        "oppT": nc.dram_tensor("oppT", [BPC, H, L], BF, kind="ExternalInput").ap(),
        "WqT": nc.dram_tensor("WqT", [H, H], BF, kind="ExternalInput").ap(),
        "Wk": nc.dram_tensor("Wk", [H, H], BF, kind="ExternalInput").ap(),
        "WvT": nc.dram_tensor("WvT", [H, H], BF, kind="ExternalInput").ap(),
        "bq_p": nc.dram_tensor("bq_p", [P, HT], F32, kind="ExternalInput").ap(),
        "bv_b": nc.dram_tensor("bv_b", [P, H], F32, kind="ExternalInput").ap(),
    }
    outs = {
        "self_pv": nc.dram_tensor("self_pv", [BPC, L, H], BF,
                                  kind="ExternalOutput").ap(),
        "opp_pv": nc.dram_tensor("opp_pv", [BPC, L, H], BF,
                                 kind="ExternalOutput").ap(),
        "rs": nc.dram_tensor("rs", [BPC, 2, L], F32,
                             kind="ExternalOutput").ap(),
    }
    with tile.TileContext(nc) as tc:
        with contextlib.ExitStack() as ctx:
            _build_core_kernel(ctx, tc, ins, outs)
    nc.compile()
    _NC_CACHE = nc
    return nc


def kernel(q, opp, Wq, bq, Wk, bk, Wv, bv):
    q = np.asarray(q, dtype=np.float32)
    opp = np.asarray(opp, dtype=np.float32)
    Wq = np.asarray(Wq, dtype=np.float32)
    Wk = np.asarray(Wk, dtype=np.float32)
    Wv = np.asarray(Wv, dtype=np.float32)
    bq = np.asarray(bq, dtype=np.float32)
    bv = np.asarray(bv, dtype=np.float32)
    # bk is mathematically irrelevant (softmax shift-invariance); unused.

    bf = ml_dtypes.bfloat16
    qT = np.ascontiguousarray(q.transpose(0, 2, 1)).astype(bf)    # [B, H, L]
    oppT = np.ascontiguousarray(opp.transpose(0, 2, 1)).astype(bf)
    shared = {
        "WqT": np.ascontiguousarray(Wq.T).astype(bf),
        "Wk": np.ascontiguousarray(Wk).astype(bf),
        "WvT": np.ascontiguousarray(Wv.T).astype(bf),
        "bq_p": np.ascontiguousarray(bq.reshape(HT, P).T),
        "bv_b": np.ascontiguousarray(np.tile(bv, (P, 1))),
    }
    in_maps = []
    for c in range(NCORES):
        sl = slice(c * BPC, (c + 1) * BPC)
        in_maps.append({
            "qT": np.ascontiguousarray(qT[sl]),
            "oppT": np.ascontiguousarray(oppT[sl]),
            **shared,
        })

    nc = _get_module()
    res = run_bass_kernel_spmd(nc, in_maps, core_ids=list(range(NCORES)))
    self_pv = np.concatenate([r["self_pv"] for r in res.results], axis=0)
    opp_pv = np.concatenate([r["opp_pv"] for r in res.results], axis=0)
    rs = np.concatenate([r["rs"] for r in res.results], axis=0)  # [B, 2, L]
    self_out = self_pv.astype(np.float32) / rs[:, 0, :, None]
    opp_out = opp_pv.astype(np.float32) / rs[:, 1, :, None]
    return (self_out, opp_out)
